# Initial kernel scaffold
#
# Trainium2 Bass kernel for the BronxLayer GNN message-passing problem.
#
# Reference math (fp32):
#   hn = LayerNorm(h)*gamma + beta ; xn = x / max(|x|_1, 1e-12)
#   k = hn@w_k.T ; q = hn@w_q.T ; a_h = softmax(k@q.T/16) ; a_x = xn@xn.T
#   i = [diag(a_x), rowsum(a_x), rowstd(a_x, ddof=1)] ; m = softmax(mixing, 0)
#   x_out = (m00*a_x + m10*a_h)@xn + x
#   h_agg = m01*(a_x@hn) + m11*(a_h.T@hn)          (a_x symmetric)
#   h_out = elu([h_agg|i]@w_v.T) + h
#
# Sharding: nodes row-sharded over 8 cores (512 rows each). Per core:
#   - replicated preprocessing (LN, L1, transposes, qT, Gram matrix)
#   - row block of S -> E = exp(S/16) (softmax normalization folded into
#     downstream scales via 1/rowsum)
#   - a_x column blocks computed directly by matmul (symmetry)
#   - the only cross-core term, m11*(a_h.T@hn), is formed as
#     partial = E_rows.T @ [hn_loc*m11/rowsum | m11/rowsum] per core and
#     summed with one ReduceScatter that hands each core its own row block
#     (the extra column carries the a_h column sums for the beta term).
#
# gamma/beta are applied only in transposed (feature-on-partition) layouts
# where they are per-partition ACT scale/bias: exactly on hnT (k/q path),
# as a column scale on h_aggT, and the remaining beta contribution
# beta[f]*colsum(a_h2)[m] enters the w_v matmul as one extra contraction row.
import sys

if "/opt/trn_rl_repo" not in sys.path:
    sys.path.insert(0, "/opt/trn_rl_repo")

import numpy as np

N, F = 4096, 256
NCORES = 8
R = N // NCORES  # 512
P = 128
MT = R // P      # 4
NT = N // P      # 32
FT = F // P      # 2
NCH = N // 512   # 8
FP = F + 8       # partial width: hn cols + colsum col + pad (32B-aligned rows)
LN_EPS = 1e-5
L1_EPS = 1e-12
SCALE = float(1.0 / np.sqrt(F))

_CACHE = {}


def _build():
    import contextlib

    import concourse.mybir as mybir
    import concourse.tile as tile
    from concourse import bacc
    from concourse.bass import ds, ts
    from concourse.masks import make_identity

    f32 = mybir.dt.float32
    f32r = mybir.dt.float32r
    bf16 = mybir.dt.bfloat16
    AF = mybir.ActivationFunctionType
    OP = mybir.AluOpType
    AX = mybir.AxisListType

    nc = bacc.Bacc(None, num_devices=NCORES)

    h_ext = nc.declare_dram_parameter("h", [N, F], f32, isOutput=False)
    x_ext = nc.declare_dram_parameter("x", [N, F], f32, isOutput=False)
    hloc_ext = nc.declare_dram_parameter("h_loc", [R, F], f32, isOutput=False)
    xloc_ext = nc.declare_dram_parameter("x_loc", [R, F], f32, isOutput=False)
    wkT_ext = nc.declare_dram_parameter("w_kT", [F, F], f32, isOutput=False)
    wqT_ext = nc.declare_dram_parameter("w_qT", [F, F], f32, isOutput=False)
    wvT_ext = nc.declare_dram_parameter("w_vT", [F + 3, F], f32r, isOutput=False)
    mix_ext = nc.declare_dram_parameter("mixing", [2, 2], f32, isOutput=False)
    gam_ext = nc.declare_dram_parameter("ln_gamma", [F], f32, isOutput=False)
    bet_ext = nc.declare_dram_parameter("ln_beta", [F], f32, isOutput=False)
    hout_ext = nc.declare_dram_parameter("h_out", [R, F], f32, isOutput=True)
    xout_ext = nc.declare_dram_parameter("x_out", [R, F], f32, isOutput=True)

    h_v = h_ext.rearrange("(o p) f -> p o f", p=P)
    x_v = x_ext.rearrange("(o p) f -> p o f", p=P)
    hloc_v = hloc_ext.rearrange("(o p) f -> p o f", p=P)
    xloc_v = xloc_ext.rearrange("(o p) f -> p o f", p=P)
    hout_v = hout_ext.rearrange("(o p) f -> p o f", p=P)
    xout_v = xout_ext.rearrange("(o p) f -> p o f", p=P)

    with tile.TileContext(nc) as tc, contextlib.ExitStack() as ctx:
        const = ctx.enter_context(tc.tile_pool(name="const", bufs=1))
        persist = ctx.enter_context(tc.tile_pool(name="persist", bufs=1))
        dram = ctx.enter_context(tc.tile_pool(name="dram", bufs=1, space="DRAM"))
        stream = ctx.enter_context(tc.tile_pool(name="stream", bufs=4))
        small = ctx.enter_context(tc.tile_pool(name="small", bufs=3))

        # ---------------- constants ----------------
        ident_f = const.tile([P, P], f32, name="ident_f")
        make_identity(nc, ident_f)
        ident_b = const.tile([P, P], bf16, name="ident_b")
        make_identity(nc, ident_b)
        eps_ln = const.tile([P, 1], f32, name="eps_ln")
        nc.vector.memset(eps_ln[:], LN_EPS)
        # gamma/beta in feature-on-partition layout [P, FT, 1]
        gam_f = const.tile([P, FT, 1], f32, name="gam_f")
        nc.sync.dma_start(gam_f[:, :, 0], gam_ext.rearrange("(o p) -> p o", p=P))
        bet_f = const.tile([P, FT, 1], f32, name="bet_f")
        nc.sync.dma_start(bet_f[:, :, 0], bet_ext.rearrange("(o p) -> p o", p=P))
        # w_k.T / w_q.T as bf16 [fi, fo] (staged through f32)
        wk_st = stream.tile([P, FT, F], f32, name="wk_st", tag="w_st", bufs=1)
        nc.sync.dma_start(wk_st[:], wkT_ext.rearrange("(o p) f -> p o f", p=P))
        wkT = const.tile([P, FT, F], bf16, name="wkT")
        nc.vector.tensor_copy(out=wkT[:], in_=wk_st[:])
        wq_st = stream.tile([P, FT, F], f32, name="wq_st", tag="w_st2", bufs=1)
        nc.sync.dma_start(wq_st[:], wqT_ext.rearrange("(o p) f -> p o f", p=P))
        wqT = const.tile([P, FT, F], bf16, name="wqT")
        nc.vector.tensor_copy(out=wqT[:], in_=wq_st[:])
        # w_v.T rows 0..255 (f32r) + padded tail: rows 0..2 = w_v cols 256..258,
        # row 3 = beta @ w_v[:, :F].T (rank-1 beta term), rest zero
        wvT = const.tile([P, FT, F], f32r, name="wvT")
        nc.sync.dma_start(wvT[:], wvT_ext[:F].rearrange("(o p) f -> p o f", p=P))
        wvT3 = const.tile([P, F], f32r, name="wvT3")
        nc.vector.memset(wvT3[:].bitcast(f32), 0.0)
        bet_pad = const.tile([P, FT, 4], f32r, name="bet_pad")
        nc.vector.memset(bet_pad[:].bitcast(f32), 0.0)
        nc.vector.tensor_copy(out=bet_pad[:, :, 3:4], in_=bet_f[:])

        # m = softmax(mixing, axis=0); flat order [m00, m01, m10, m11]
        m_flat = const.tile([1, 4], f32, name="m_flat")
        nc.sync.dma_start(m_flat[:], mix_ext.rearrange("a b -> () (a b)"))
        m_exp = const.tile([1, 4], f32, name="m_exp")
        nc.scalar.activation(m_exp[:], m_flat[:], AF.Exp)
        m_cs = const.tile([1, 2], f32, name="m_cs")
        nc.vector.tensor_tensor(m_cs[:], m_exp[:, 0:2], m_exp[:, 2:4], OP.add)
        m_rc = const.tile([1, 2], f32, name="m_rc")
        nc.vector.reciprocal(m_rc[:], m_cs[:])
        m_n = const.tile([1, 4], f32, name="m_n")
        nc.vector.tensor_tensor(m_n[:, 0:2], m_exp[:, 0:2], m_rc[:], OP.mult)
        nc.vector.tensor_tensor(m_n[:, 2:4], m_exp[:, 2:4], m_rc[:], OP.mult)
        m_dram = dram.tile([1, 4], f32, name="m_dram")
        nc.sync.dma_start(m_dram[:], m_n[:])
        m_bc = const.tile([P, 4], f32, name="m_bc")
        nc.sync.dma_start(m_bc[:], m_dram[:].to_broadcast((P, 4)))
        M00, M01, M10, M11 = (m_bc[:, j : j + 1] for j in range(4))
        rm01 = const.tile([P, 1], f32, name="rm01")
        nc.vector.reciprocal(rm01[:], M01)
        gam_eff = const.tile([P, FT, 1], f32, name="gam_eff")
        for _ft in range(FT):
            nc.vector.tensor_tensor(gam_eff[:, _ft], gam_f[:, _ft], rm01[:], OP.mult)

        # ---------------- persistent tensors ----------------
        qT = persist.tile([P, FT, N], bf16, name="qT")
        xnT = persist.tile([P, FT, N], bf16, name="xnT")
        xh_b = persist.tile([P, NT, 2 * F], bf16, name="xh_b")
        E = persist.tile([P, MT, N], bf16, name="E")
        kT_loc = persist.tile([P, FT, R], bf16, name="kT_loc")
        xnT_loc = persist.tile([P, FT, R], bf16, name="xnT_loc")
        hn_loc = persist.tile([P, MT, F], bf16, name="hn_loc")
        hn_scaled = persist.tile([P, MT, FP], bf16, name="hn_scaled")
        xn_loc_b = persist.tile([P, MT, F], bf16, name="xn_loc_b")
        G_b = persist.tile([P, FT, F], bf16, name="G_b")
        GXT = persist.tile([P, MT, F], bf16, name="GXT")
        rowsum_parts = persist.tile([P, MT, NCH], f32, name="rowsum_parts")
        recip_r = persist.tile([P, MT, 1], f32, name="recip_r")
        diag = persist.tile([P, MT, 1], f32, name="diag")
        srow = persist.tile([P, MT, 1], f32, name="srow")
        stdv = persist.tile([P, MT, 1], f32, name="stdv")
        sumsq = persist.tile([P, MT, 1], f32, name="sumsq")
        s_f = persist.tile([P, FT, 1], f32, name="s_f")
        s_b = persist.tile([P, FT, 1], bf16, name="s_b")
        i_cols = persist.tile([P, MT, 4], f32, name="i_cols")
        i_T = persist.tile([P, R], f32r, name="i_T")
        rs_sb = persist.tile([P, MT, FP], f32, name="rs_sb")
        h_agg = persist.tile([P, MT, F], f32, name="h_agg")
        h_aggT = persist.tile([P, FT, R], f32r, name="h_aggT")

        partial_dram = dram.tile([N, FP], f32, name="partial_dram")
        rs_dram = dram.tile([R, FP], f32, name="rs_dram")

        # ============ phase 1: preprocessing + S/E ============
        with tc.tile_pool(name="p1", bufs=1, space="PSUM") as p1, \
             tc.tile_pool(name="sc1", bufs=1) as sc1:

            # bvec = beta @ w_v[:, :F].T into row 3 of wvT3 (rows 0..2 zero),
            # then the w_v tail rows overwrite rows 0..2.
            ps_bv = p1.tile([4, F], f32, name="ps_bv", tag="mm", bufs=4)
            for k in range(FT):
                nc.tensor.matmul(
                    ps_bv[:],
                    bet_pad[:, k],
                    wvT[:, k],
                    start=(k == 0),
                    stop=(k == FT - 1),
                )
            nc.vector.tensor_copy(out=wvT3[:4, :], in_=ps_bv[:])
            nc.sync.dma_start(wvT3[:3, :], wvT_ext[F:])

            # ---- local rows: batched LN/L1 stats ----
            hl_in = sc1.tile([P, MT, F], f32, name="hl_in")
            nc.sync.dma_start(hl_in[:], hloc_v[:])
            xl_in = sc1.tile([P, MT, F], f32, name="xl_in")
            nc.sync.dma_start(xl_in[:], xloc_v[:])
            st6l = small.tile([P, MT, 6], f32, name="st6l", tag="st6b")
            for j in range(MT):
                nc.vector.bn_stats(st6l[:, j], hl_in[:, j])
            mvl = small.tile([P, MT, 2], f32, name="mvl", tag="mvb")
            for j in range(MT):
                nc.vector.bn_aggr(mvl[:, j], st6l[:, j])
            sdl = small.tile([P, MT], f32, name="sdl", tag="sdb")
            nc.scalar.activation(sdl[:], mvl[:, :, 1], AF.Sqrt, bias=eps_ln[:])
            rstdl = small.tile([P, MT], f32, name="rstdl", tag="rstdb")
            nc.vector.reciprocal(rstdl[:], sdl[:])
            nmrl = small.tile([P, MT], f32, name="nmrl", tag="nmrb")
            nc.vector.tensor_tensor(nmrl[:], mvl[:, :, 0], rstdl[:], OP.mult)
            nc.vector.tensor_scalar_mul(nmrl[:], nmrl[:], -1.0)
            l1l = small.tile([P, MT], f32, name="l1l", tag="l1b")
            nc.vector.tensor_reduce(
                l1l[:], xl_in[:], AX.X, OP.add, apply_absolute_value=True
            )
            nc.vector.tensor_scalar_max(l1l[:], l1l[:], L1_EPS)
            rl1l = small.tile([P, MT], f32, name="rl1l", tag="rl1b")
            nc.vector.reciprocal(rl1l[:], l1l[:])
            for mt in range(MT):
                nc.scalar.activation(
                    hn_loc[:, mt], hl_in[:, mt], AF.Identity,
                    bias=nmrl[:, mt : mt + 1], scale=rstdl[:, mt : mt + 1],
                )
                nc.scalar.activation(
                    xn_loc_b[:, mt], xl_in[:, mt], AF.Copy,
                    scale=rl1l[:, mt : mt + 1],
                )
                # diag(a_x)[m] = ||x_m||^2 / l1^2
                sq = small.tile([P, F], f32, name="sq", tag="sq", bufs=2)
                nc.scalar.activation(sq[:], xl_in[:, mt], AF.Square)
                ssq = small.tile([P, 1], f32, name="ssq", tag="ssq")
                nc.vector.tensor_reduce(ssq[:], sq[:], AX.X, OP.add)
                rl1sq = small.tile([P, 1], f32, name="rl1sq", tag="rl1sq")
                nc.scalar.activation(
                    rl1sq[:], rl1l[:, mt : mt + 1], AF.Square
                )
                nc.vector.tensor_tensor(diag[:, mt], ssq[:], rl1sq[:], OP.mult)

            # local transposes (bf16) + kT_loc
            hnT_l = sc1.tile([P, FT, R], bf16, name="hnT_l")
            for ft in range(FT):
                ps_tb = p1.tile([P, 512], bf16, name="ps_tb", tag="tp_b", bufs=2)
                for mt in range(MT):
                    nc.tensor.transpose(
                        ps_tb[:, ts(mt, P)], hn_loc[:, mt, ds(128 * ft, P)], ident_b[:]
                    )
                # gamma/beta are per-partition in this layout
                nc.scalar.activation(
                    hnT_l[:, ft], ps_tb[:], AF.Identity,
                    bias=bet_f[:, ft], scale=gam_f[:, ft],
                )
            for ft in range(FT):
                ps_tb = p1.tile([P, 512], bf16, name="ps_tb2", tag="tp_b", bufs=2)
                for mt in range(MT):
                    nc.tensor.transpose(
                        ps_tb[:, ts(mt, P)], xn_loc_b[:, mt, ds(128 * ft, P)], ident_b[:]
                    )
                nc.vector.tensor_copy(out=xnT_loc[:, ft], in_=ps_tb[:])
            for fo in range(FT):
                ps_k = p1.tile([P, 512], f32, name="ps_k", tag="mm", bufs=4)
                for k in range(FT):
                    nc.tensor.matmul(
                        ps_k[:],
                        wkT[:, k, ds(128 * fo, P)],
                        hnT_l[:, k],
                        start=(k == 0),
                        stop=(k == FT - 1),
                    )
                nc.vector.tensor_copy(out=kT_loc[:, fo], in_=ps_k[:])

            # ---- fused h+x per chunk: LN/L1 -> hnT/xnT -> qT -> S/E -> G ----
            ps_G = p1.tile([P, 2 * F], f32, name="ps_G", tag="gacc", bufs=1)
            for c in range(NCH):
                x_in = stream.tile([P, 4, F], f32, name="x_in", tag="hx_in")
                nc.sync.dma_start(x_in[:], x_v[:, ds(4 * c, 4)])
                l1b = small.tile([P, 4], f32, name="l1x", tag="l1b")
                nc.vector.tensor_reduce(
                    l1b[:], x_in[:], AX.X, OP.add, apply_absolute_value=True
                )
                nc.vector.tensor_scalar_max(l1b[:], l1b[:], L1_EPS)
                rl1b = small.tile([P, 4], f32, name="rl1x", tag="rl1b")
                nc.vector.reciprocal(rl1b[:], l1b[:])
                for j in range(4):
                    nt = 4 * c + j
                    nc.vector.tensor_scalar_mul(
                        xh_b[:, nt, 0:F], x_in[:, j], rl1b[:, j : j + 1]
                    )
                for ft in range(FT):
                    ps_tb = p1.tile([P, 512], bf16, name="ps_tb4", tag="tp_b", bufs=2)
                    for j in range(4):
                        nt = 4 * c + j
                        nc.tensor.transpose(
                            ps_tb[:, ts(j, P)], xh_b[:, nt, ds(128 * ft, P)], ident_b[:]
                        )
                    nc.vector.tensor_copy(out=xnT[:, ft, ds(512 * c, 512)], in_=ps_tb[:])
                for j in range(4):
                    nt = 4 * c + j
                    for m in range(FT):
                        nc.tensor.matmul(
                            ps_G[:, ts(m, F)],
                            xh_b[:, nt, ds(128 * m, P)],
                            xh_b[:, nt, 0:F],
                            start=(nt == 0),
                            stop=(nt == NT - 1),
                        )
                if c == NCH - 1:
                    for m in range(FT):
                        nc.scalar.activation(G_b[:, m], ps_G[:, ts(m, F)], AF.Copy)

                h_in = stream.tile([P, 4, F], f32, name="h_in", tag="hx_in")
                nc.sync.dma_start(h_in[:], h_v[:, ds(4 * c, 4)])
                st6 = small.tile([P, 4, 6], f32, name="st6h", tag="st6b")
                for j in range(4):
                    nc.vector.bn_stats(st6[:, j], h_in[:, j])
                mvb = small.tile([P, 4, 2], f32, name="mvb", tag="mvb")
                for j in range(4):
                    nc.vector.bn_aggr(mvb[:, j], st6[:, j])
                sdb = small.tile([P, 4], f32, name="sdb", tag="sdb")
                nc.scalar.activation(sdb[:], mvb[:, :, 1], AF.Sqrt, bias=eps_ln[:])
                rstdb = small.tile([P, 4], f32, name="rstdb", tag="rstdb")
                nc.vector.reciprocal(rstdb[:], sdb[:])
                nmrb = small.tile([P, 4], f32, name="nmrb", tag="nmrb")
                nc.vector.tensor_tensor(nmrb[:], mvb[:, :, 0], rstdb[:], OP.mult)
                nc.vector.tensor_scalar_mul(nmrb[:], nmrb[:], -1.0)
                rstm = small.tile([P, 4], f32, name="rstm", tag="rstm")
                nc.vector.tensor_tensor(
                    rstm[:], rstdb[:], M01.to_broadcast((P, 4)), OP.mult
                )
                nmrm = small.tile([P, 4], f32, name="nmrm", tag="nmrm")
                nc.vector.tensor_tensor(
                    nmrm[:], nmrb[:], M01.to_broadcast((P, 4)), OP.mult
                )
                for j in range(4):
                    nt = 4 * c + j
                    nc.scalar.activation(
                        xh_b[:, nt, F : 2 * F], h_in[:, j], AF.Identity,
                        bias=nmrm[:, j : j + 1], scale=rstm[:, j : j + 1],
                    )
                hnT_c = sc1.tile([P, FT, 512], bf16, name="hnT_c", tag="hnT_c", bufs=3)
                for ft in range(FT):
                    ps_tb = p1.tile([P, 512], bf16, name="ps_tb3", tag="tp_b", bufs=2)
                    for j in range(4):
                        nt = 4 * c + j
                        nc.tensor.transpose(
                            ps_tb[:, ts(j, P)],
                            xh_b[:, nt, ds(F + 128 * ft, P)],
                            ident_b[:],
                        )
                    nc.vector.tensor_scalar(
                        hnT_c[:, ft], ps_tb[:],
                        scalar1=gam_eff[:, ft], scalar2=bet_f[:, ft],
                        op0=OP.mult, op1=OP.add,
                    )
                for fo in range(FT):
                    ps_q = p1.tile([P, 512], f32, name="ps_q", tag="mm", bufs=4)
                    for k in range(FT):
                        nc.tensor.matmul(
                            ps_q[:],
                            wqT[:, k, ds(128 * fo, P)],
                            hnT_c[:, k],
                            start=(k == 0),
                            stop=(k == FT - 1),
                        )
                    nc.vector.tensor_copy(out=qT[:, fo, ds(512 * c, 512)], in_=ps_q[:])
                # S rows for this chunk, all four local m-tiles
                for mt in range(MT):
                    ps_s = p1.tile([P, 512], f32, name="ps_s", tag="mm", bufs=4)
                    for k in range(FT):
                        nc.tensor.matmul(
                            ps_s[:],
                            kT_loc[:, k, ds(128 * mt, P)],
                            qT[:, k, ds(512 * c, 512)],
                            start=(k == 0),
                            stop=(k == FT - 1),
                        )
                    nc.scalar.activation(
                        E[:, mt, ds(512 * c, 512)],
                        ps_s[:],
                        AF.Exp,
                        scale=SCALE,
                        accum_out=rowsum_parts[:, mt, c : c + 1],
                    )

            # rowsums -> 1/rowsum -> hn_scaled
            for mt in range(MT):
                rs1 = small.tile([P, 1], f32, name="rs1", tag="rs1")
                nc.vector.tensor_reduce(rs1[:], rowsum_parts[:, mt], AX.X, OP.add)
                nc.vector.reciprocal(recip_r[:, mt], rs1[:])
                sch = small.tile([P, 1], f32, name="sch", tag="sch")
                nc.vector.tensor_tensor(sch[:], recip_r[:, mt], M11, OP.mult)
                nc.scalar.activation(
                    hn_scaled[:, mt, 0:F], hn_loc[:, mt], AF.Copy, scale=sch[:]
                )
                nc.vector.memset(hn_scaled[:, mt, F:FP], 0.0)
                nc.vector.tensor_copy(out=hn_scaled[:, mt, F : F + 1], in_=sch[:])

        # ============ phase 2: a_x row stats + bvec ============
        with tc.tile_pool(name="p2", bufs=1, space="PSUM") as p2, \
             tc.tile_pool(name="sc2", bufs=1) as sc2:
            for ft in range(FT):
                nc.vector.tensor_reduce(s_f[:, ft], xnT[:, ft], AX.X, OP.add)
                nc.vector.tensor_copy(out=s_b[:, ft], in_=s_f[:, ft])
            ps_sr = p2.tile([P, MT], f32, name="ps_sr", tag="sr", bufs=1)
            for mt in range(MT):
                for k in range(FT):
                    nc.tensor.matmul(
                        ps_sr[:, mt : mt + 1],
                        xnT_loc[:, k, ds(128 * mt, P)],
                        s_b[:, k],
                        start=(k == 0),
                        stop=(k == FT - 1),
                    )
            nc.vector.tensor_copy(out=srow[:, :, 0], in_=ps_sr[:])

            GX_sb = sc2.tile([P, FT, R], bf16, name="GX_sb")
            for f1 in range(FT):
                ps_gx = p2.tile([P, 512], f32, name="ps_gx", tag="gx", bufs=2)
                for k in range(FT):
                    nc.tensor.matmul(
                        ps_gx[:],
                        G_b[:, k, ds(128 * f1, P)],
                        xnT_loc[:, k],
                        start=(k == 0),
                        stop=(k == FT - 1),
                    )
                nc.vector.tensor_copy(out=GX_sb[:, f1], in_=ps_gx[:])
            for mt in range(MT):
                ps_gxt = p2.tile([P, F], bf16, name="ps_gxt", tag="gxt", bufs=2)
                for f1 in range(FT):
                    nc.tensor.transpose(
                        ps_gxt[:, ts(f1, P)], GX_sb[:, f1, ds(128 * mt, P)], ident_b[:]
                    )
                nc.vector.tensor_copy(out=GXT[:, mt], in_=ps_gxt[:])
            for mt in range(MT):
                tmp = small.tile([P, F], f32, name="tmp_ss", tag="sq", bufs=2)
                nc.vector.tensor_tensor(tmp[:], xn_loc_b[:, mt], GXT[:, mt], OP.mult)
                nc.vector.tensor_reduce(sumsq[:, mt], tmp[:], AX.X, OP.add)
                t1 = small.tile([P, 1], f32, name="t1", tag="t1")
                nc.scalar.activation(t1[:], srow[:, mt], AF.Square)
                nc.vector.tensor_scalar_mul(t1[:], t1[:], -1.0 / N)
                nc.vector.tensor_tensor(t1[:], sumsq[:, mt], t1[:], OP.add)
                nc.vector.tensor_scalar_mul(t1[:], t1[:], 1.0 / (N - 1))
                nc.scalar.activation(stdv[:, mt], t1[:], AF.Sqrt)

        # ============ phase 3: ET/m10, partial+RS, a_xc, combines ============
        with tc.tile_pool(name="pL", bufs=1, space="PSUM") as pL, \
             tc.tile_pool(name="sc3", bufs=1) as sc3:
            ps_b01 = pL.tile([P, 512], f32, name="ps_b01", tag="b01", bufs=1)
            ps_b23 = pL.tile([P, 512], f32, name="ps_b23", tag="b23", bufs=1)
            bslc = [ps_b01[:, 0:F], ps_b01[:, F:], ps_b23[:, 0:F], ps_b23[:, F:]]

            # ---- partial = E.T @ [hn_scaled | m11/rowsum] -> DRAM ----
            for ic in range(NT):
                ps_p = pL.tile([P, FP], f32, name="ps_p", tag="w", bufs=2)
                for jt in range(MT):
                    nc.tensor.matmul(
                        ps_p[:],
                        E[:, jt, ds(128 * ic, P)],
                        hn_scaled[:, jt],
                        start=(jt == 0),
                        stop=(jt == MT - 1),
                    )
                stg = sc3.tile([P, FP], f32, name="stg", tag="stg", bufs=3)
                nc.vector.tensor_copy(out=stg[:], in_=ps_p[:])
                nc.sync.dma_start(
                    partial_dram.rearrange("(a p) f -> p a f", p=P)[:, ic], stg[:]
                )
            # ---- ET transposes + x_out m10 accumulation ----
            for mt in range(MT):
                for c in range(NCH):
                    ps_et = pL.tile([P, 512], bf16, name="ps_et", tag="w", bufs=2)
                    for j in range(4):
                        nt = 4 * c + j
                        nc.tensor.transpose(
                            ps_et[:, ts(j, P)], E[:, mt, ds(128 * nt, P)], ident_b[:]
                        )
                    ET_c = sc3.tile([P, 4, P], bf16, name="ET_c", tag="ET", bufs=3)
                    nc.vector.tensor_copy(out=ET_c[:], in_=ps_et[:])
                    for j in range(4):
                        nt = 4 * c + j
                        nc.tensor.matmul(
                            bslc[mt],
                            ET_c[:, j],
                            xh_b[:, nt, 0:F],
                            start=(nt == 0),
                            stop=(nt == NT - 1),
                        )

            # ---- a_xc streaming + packed [x00 | SYM] accumulation ----
            ps_xs = [
                pL.tile([P, 512], f32, name=f"ps_xs{mt}", tag=f"xs{mt}", bufs=1)
                for mt in range(MT)
            ]
            for nt in range(NT):
                ps_ax = pL.tile([P, 512], f32, name="ps_ax", tag="w", bufs=2)
                for k in range(FT):
                    nc.tensor.matmul(
                        ps_ax[:],
                        xnT[:, k, ds(128 * nt, P)],
                        xnT_loc[:, k],
                        start=(k == 0),
                        stop=(k == FT - 1),
                    )
                axc = sc3.tile([P, 512], bf16, name="axc", tag="axc", bufs=3)
                nc.scalar.activation(axc[:], ps_ax[:], AF.Copy)
                for mt in range(MT):
                    nc.tensor.matmul(
                        ps_xs[mt],
                        axc[:, ds(128 * mt, P)],
                        xh_b[:, nt],
                        start=(nt == 0),
                        stop=(nt == NT - 1),
                    )

            nc.gpsimd.collective_compute(
                "ReduceScatter",
                OP.add,
                replica_groups=[list(range(NCORES))],
                ins=[partial_dram[:]],
                outs=[rs_dram[:]],
            )
            nc.gpsimd.dma_start(rs_sb[:], rs_dram.rearrange("(o p) f -> p o f", p=P))

            # ---- x_out combine ----
            for mt in range(MT):
                xo = small.tile([P, F], f32, name="xo", tag="xo", bufs=2)
                nc.vector.tensor_scalar_mul(xo[:], ps_xs[mt][:, 0:F], M00)
                scb = small.tile([P, 1], f32, name="scb", tag="scb")
                nc.vector.tensor_tensor(scb[:], recip_r[:, mt], M10, OP.mult)
                tb = small.tile([P, F], f32, name="tb", tag="tb", bufs=2)
                nc.scalar.activation(tb[:], bslc[mt], AF.Copy, scale=scb[:])
                nc.vector.tensor_tensor(xo[:], xo[:], tb[:], OP.add)
                xr = stream.tile([P, F], f32, name="xr", tag="res")
                nc.sync.dma_start(xr[:], xloc_v[:, mt])
                nc.vector.tensor_tensor(xo[:], xo[:], xr[:], OP.add)
                nc.sync.dma_start(xout_v[:, mt], xo[:])

            # ---- h path ----
            # i rows: [diag, srow, std, m01*srow + rs_extra] ; h_agg = SYM + rs
            for mt in range(MT):
                nc.gpsimd.tensor_copy(out=i_cols[:, mt, 0:1], in_=diag[:, mt])
                nc.gpsimd.tensor_copy(out=i_cols[:, mt, 1:2], in_=srow[:, mt])
                nc.gpsimd.tensor_copy(out=i_cols[:, mt, 2:3], in_=stdv[:, mt])
                c4 = small.tile([P, 1], f32, name="c4", tag="c4", bufs=4)
                nc.gpsimd.tensor_tensor(c4[:], srow[:, mt], M01, OP.mult)
                nc.gpsimd.tensor_tensor(c4[:], c4[:], rs_sb[:, mt, F : F + 1], OP.add)
                nc.gpsimd.tensor_copy(out=i_cols[:, mt, 3:4], in_=c4[:])
                sym_sb = small.tile([P, F], f32, name="sym_sb", tag="sym", bufs=4)
                nc.vector.tensor_copy(out=sym_sb[:], in_=ps_xs[mt][:, F:])
                nc.gpsimd.tensor_tensor(
                    h_agg[:, mt], sym_sb[:], rs_sb[:, mt, 0:F], OP.add
                )
            nc.vector.memset(i_T[:].bitcast(f32), 0.0)
            for mt in range(MT):
                ps_i = pL.tile([4, P], f32, name="ps_i", tag="w", bufs=2)
                nc.tensor.transpose(ps_i[:], i_cols[:, mt], ident_f[:])
                nc.vector.tensor_copy(out=i_T[:4, ds(128 * mt, P)], in_=ps_i[:])
            for ft in range(FT):
                ps_hat = pL.tile([P, 512], f32, name="ps_hat", tag="w", bufs=2)
                for mt in range(MT):
                    nc.tensor.transpose(
                        ps_hat[:, ts(mt, P)], h_agg[:, mt, ds(128 * ft, P)], ident_f[:]
                    )
                # gamma is a per-partition column scale here
                nc.scalar.activation(h_aggT[:, ft], ps_hat[:], AF.Copy, scale=gam_f[:, ft])
            for mt in range(MT):
                ps_h = pL.tile([P, F], f32, name="ps_h", tag="w", bufs=2)
                for k in range(FT):
                    nc.tensor.matmul(
                        ps_h[:],
                        h_aggT[:, k, ds(128 * mt, P)],
                        wvT[:, k],
                        start=(k == 0),
                        stop=False,
                    )
                nc.tensor.matmul(
                    ps_h[:],
                    i_T[:, ds(128 * mt, P)],
                    wvT3[:],
                    start=False,
                    stop=True,
                )
                vmin = small.tile([P, F], f32, name="vmin", tag="vmin", bufs=2)
                nc.vector.tensor_scalar_min(vmin[:], ps_h[:], 0.0)
                ev = small.tile([P, F], f32, name="ev", tag="ev", bufs=2)
                nc.scalar.activation(ev[:], vmin[:], AF.Exp)
                vmax = small.tile([P, F], f32, name="vmax", tag="vmax", bufs=2)
                nc.vector.tensor_scalar_max(vmax[:], ps_h[:], 0.0)
                ho = small.tile([P, F], f32, name="ho", tag="ho", bufs=2)
                nc.vector.tensor_tensor(ho[:], ev[:], vmax[:], OP.add)
                nc.vector.tensor_scalar_add(ho[:], ho[:], -1.0)
                hr = stream.tile([P, F], f32, name="hr", tag="res")
                nc.sync.dma_start(hr[:], hloc_v[:, mt])
                nc.vector.tensor_tensor(ho[:], ho[:], hr[:], OP.add)
                nc.sync.dma_start(hout_v[:, mt], ho[:])

    nc.finalize()
    return nc


def _make_in_maps(inputs):
    h = np.ascontiguousarray(inputs["h"], dtype=np.float32)
    x = np.ascontiguousarray(inputs["x"], dtype=np.float32)
    w_kT = np.ascontiguousarray(np.asarray(inputs["w_k"], np.float32).T)
    w_qT = np.ascontiguousarray(np.asarray(inputs["w_q"], np.float32).T)
    w_vT = np.ascontiguousarray(np.asarray(inputs["w_v"], np.float32).T)
    mixing = np.ascontiguousarray(inputs["mixing"], dtype=np.float32)
    gam = np.ascontiguousarray(inputs["ln_gamma"], dtype=np.float32)
    bet = np.ascontiguousarray(inputs["ln_beta"], dtype=np.float32)
    return [
        {
            "h": h,
            "x": x,
            "h_loc": np.ascontiguousarray(h[c * R : (c + 1) * R]),
            "x_loc": np.ascontiguousarray(x[c * R : (c + 1) * R]),
            "w_kT": w_kT,
            "w_qT": w_qT,
            "w_vT": w_vT,
            "mixing": mixing,
            "ln_gamma": gam,
            "ln_beta": bet,
        }
        for c in range(NCORES)
    ]


def kernel(h, x, w_k, w_q, w_v, mixing, ln_gamma, ln_beta):
    from concourse.bass_utils import run_bass_kernel_spmd

    if "nc" not in _CACHE:
        _CACHE["nc"] = _build()
    nc = _CACHE["nc"]

    in_maps = _make_in_maps(
        {
            "h": h,
            "x": x,
            "w_k": w_k,
            "w_q": w_q,
            "w_v": w_v,
            "mixing": mixing,
            "ln_gamma": ln_gamma,
            "ln_beta": ln_beta,
        }
    )
    res = run_bass_kernel_spmd(nc, in_maps, list(range(NCORES))).results
    h_out = np.concatenate([res[c]["h_out"] for c in range(NCORES)], axis=0)
    x_out = np.concatenate([res[c]["x_out"] for c in range(NCORES)], axis=0)
    return (h_out, x_out)



# revision 1
# speedup vs baseline: 1.2006x; 1.2006x over previous
# Trainium2 Bass kernel for the BronxLayer GNN message-passing problem.
#
# Reference math (fp32):
#   hn = LayerNorm(h)*gamma + beta ; xn = x / max(|x|_1, 1e-12)
#   k = hn@w_k.T ; q = hn@w_q.T ; a_h = softmax(k@q.T/16) ; a_x = xn@xn.T
#   i = [diag(a_x), rowsum(a_x), rowstd(a_x, ddof=1)] ; m = softmax(mixing, 0)
#   x_out = (m00*a_x + m10*a_h)@xn + x
#   h_agg = m01*(a_x@hn) + m11*(a_h.T@hn)          (a_x symmetric)
#   h_out = elu([h_agg|i]@w_v.T) + h
#
# Sharding: nodes row-sharded over 8 cores (512 rows each). Per core:
#   - replicated preprocessing (LN, L1, transposes, qT, Gram matrix)
#   - row block of S -> E = exp(S/16) (softmax normalization folded into
#     downstream scales via 1/rowsum)
#   - a_x column blocks computed directly by matmul (symmetry)
#   - the only cross-core term, m11*(a_h.T@hn), is formed as
#     partial = E_rows.T @ [hn_loc*m11/rowsum | m11/rowsum] per core and
#     summed with one ReduceScatter that hands each core its own row block
#     (the extra column carries the a_h column sums for the beta term).
#
# gamma/beta are applied only in transposed (feature-on-partition) layouts
# where they are per-partition ACT scale/bias: exactly on hnT (k/q path),
# as a column scale on h_aggT, and the remaining beta contribution
# beta[f]*colsum(a_h2)[m] enters the w_v matmul as one extra contraction row.
import sys

if "/opt/trn_rl_repo" not in sys.path:
    sys.path.insert(0, "/opt/trn_rl_repo")

import numpy as np

N, F = 4096, 256
NCORES = 8
R = N // NCORES  # 512
P = 128
MT = R // P      # 4
NT = N // P      # 32
FT = F // P      # 2
NCH = N // 512   # 8
FP = F + 8       # partial width: hn cols + colsum col + pad (32B-aligned rows)
LN_EPS = 1e-5
L1_EPS = 1e-12
SCALE = float(1.0 / np.sqrt(F))

_CACHE = {}


def _build():
    import contextlib

    import concourse.mybir as mybir
    import concourse.tile as tile
    from concourse import bacc
    from concourse.bass import ds, ts
    from concourse.masks import make_identity

    f32 = mybir.dt.float32
    f32r = mybir.dt.float32r
    bf16 = mybir.dt.bfloat16
    AF = mybir.ActivationFunctionType
    OP = mybir.AluOpType
    AX = mybir.AxisListType

    nc = bacc.Bacc(None, num_devices=NCORES)

    h_ext = nc.declare_dram_parameter("h", [N, F], f32, isOutput=False)
    x_ext = nc.declare_dram_parameter("x", [N, F], f32, isOutput=False)
    hloc_ext = nc.declare_dram_parameter("h_loc", [R, F], f32, isOutput=False)
    xloc_ext = nc.declare_dram_parameter("x_loc", [R, F], f32, isOutput=False)
    wkT_ext = nc.declare_dram_parameter("w_kT", [F, F], f32, isOutput=False)
    wqT_ext = nc.declare_dram_parameter("w_qT", [F, F], f32, isOutput=False)
    wvT_ext = nc.declare_dram_parameter("w_vT", [F + 3, F], f32r, isOutput=False)
    mix_ext = nc.declare_dram_parameter("mixing", [2, 2], f32, isOutput=False)
    gam_ext = nc.declare_dram_parameter("ln_gamma", [F], f32, isOutput=False)
    bet_ext = nc.declare_dram_parameter("ln_beta", [F], f32, isOutput=False)
    hout_ext = nc.declare_dram_parameter("h_out", [R, F], f32, isOutput=True)
    xout_ext = nc.declare_dram_parameter("x_out", [R, F], f32, isOutput=True)

    h_v = h_ext.rearrange("(o p) f -> p o f", p=P)
    x_v = x_ext.rearrange("(o p) f -> p o f", p=P)
    hloc_v = hloc_ext.rearrange("(o p) f -> p o f", p=P)
    xloc_v = xloc_ext.rearrange("(o p) f -> p o f", p=P)
    hout_v = hout_ext.rearrange("(o p) f -> p o f", p=P)
    xout_v = xout_ext.rearrange("(o p) f -> p o f", p=P)

    with tile.TileContext(nc) as tc, contextlib.ExitStack() as ctx:
        const = ctx.enter_context(tc.tile_pool(name="const", bufs=1))
        persist = ctx.enter_context(tc.tile_pool(name="persist", bufs=1))
        dram = ctx.enter_context(tc.tile_pool(name="dram", bufs=1, space="DRAM"))
        stream = ctx.enter_context(tc.tile_pool(name="stream", bufs=4))
        small = ctx.enter_context(tc.tile_pool(name="small", bufs=3))

        # ---------------- constants ----------------
        ident_f = const.tile([P, P], f32, name="ident_f")
        make_identity(nc, ident_f)
        ident_b = const.tile([P, P], bf16, name="ident_b")
        make_identity(nc, ident_b)
        eps_ln = const.tile([P, 1], f32, name="eps_ln")
        nc.vector.memset(eps_ln[:], LN_EPS)
        # gamma/beta in feature-on-partition layout [P, FT, 1]
        gam_f = const.tile([P, FT, 1], f32, name="gam_f")
        nc.sync.dma_start(gam_f[:, :, 0], gam_ext.rearrange("(o p) -> p o", p=P))
        bet_f = const.tile([P, FT, 1], f32, name="bet_f")
        nc.sync.dma_start(bet_f[:, :, 0], bet_ext.rearrange("(o p) -> p o", p=P))
        # w_k.T / w_q.T as bf16 [fi, fo] (staged through f32)
        wk_st = stream.tile([P, FT, F], f32, name="wk_st", tag="w_st", bufs=1)
        nc.sync.dma_start(wk_st[:], wkT_ext.rearrange("(o p) f -> p o f", p=P))
        wkT = const.tile([P, FT, F], bf16, name="wkT")
        nc.vector.tensor_copy(out=wkT[:], in_=wk_st[:])
        wq_st = stream.tile([P, FT, F], f32, name="wq_st", tag="w_st2", bufs=1)
        nc.sync.dma_start(wq_st[:], wqT_ext.rearrange("(o p) f -> p o f", p=P))
        wqT = const.tile([P, FT, F], bf16, name="wqT")
        nc.vector.tensor_copy(out=wqT[:], in_=wq_st[:])
        # w_v.T rows 0..255 (f32r) + padded tail: rows 0..2 = w_v cols 256..258,
        # row 3 = beta @ w_v[:, :F].T (rank-1 beta term), rest zero
        wvT = const.tile([P, FT, F], f32r, name="wvT")
        nc.sync.dma_start(wvT[:], wvT_ext[:F].rearrange("(o p) f -> p o f", p=P))
        wvT3 = const.tile([P, F], f32r, name="wvT3")
        nc.vector.memset(wvT3[:].bitcast(f32), 0.0)
        bet_pad = const.tile([P, FT, 4], f32r, name="bet_pad")
        nc.vector.memset(bet_pad[:].bitcast(f32), 0.0)
        nc.vector.tensor_copy(out=bet_pad[:, :, 3:4], in_=bet_f[:])

        # m = softmax(mixing, axis=0); flat order [m00, m01, m10, m11]
        m_flat = const.tile([1, 4], f32, name="m_flat")
        nc.sync.dma_start(m_flat[:], mix_ext.rearrange("a b -> () (a b)"))
        m_exp = const.tile([1, 4], f32, name="m_exp")
        nc.scalar.activation(m_exp[:], m_flat[:], AF.Exp)
        m_cs = const.tile([1, 2], f32, name="m_cs")
        nc.vector.tensor_tensor(m_cs[:], m_exp[:, 0:2], m_exp[:, 2:4], OP.add)
        m_rc = const.tile([1, 2], f32, name="m_rc")
        nc.vector.reciprocal(m_rc[:], m_cs[:])
        m_n = const.tile([1, 4], f32, name="m_n")
        nc.vector.tensor_tensor(m_n[:, 0:2], m_exp[:, 0:2], m_rc[:], OP.mult)
        nc.vector.tensor_tensor(m_n[:, 2:4], m_exp[:, 2:4], m_rc[:], OP.mult)
        m_dram = dram.tile([1, 4], f32, name="m_dram")
        nc.sync.dma_start(m_dram[:], m_n[:])
        m_bc = const.tile([P, 4], f32, name="m_bc")
        nc.sync.dma_start(m_bc[:], m_dram[:].to_broadcast((P, 4)))
        M00, M01, M10, M11 = (m_bc[:, j : j + 1] for j in range(4))
        rm01 = const.tile([P, 1], f32, name="rm01")
        nc.vector.reciprocal(rm01[:], M01)
        gam_eff = const.tile([P, FT, 1], f32, name="gam_eff")
        for _ft in range(FT):
            nc.vector.tensor_tensor(gam_eff[:, _ft], gam_f[:, _ft], rm01[:], OP.mult)

        # ---------------- persistent tensors ----------------
        qT = persist.tile([P, FT, N], bf16, name="qT")
        xnT = persist.tile([P, FT, N], bf16, name="xnT")
        xh_b = persist.tile([P, NT, 2 * F], bf16, name="xh_b")
        E = persist.tile([P, MT, N], bf16, name="E")
        kT_loc = persist.tile([P, FT, R], bf16, name="kT_loc")
        xnT_loc = persist.tile([P, FT, R], bf16, name="xnT_loc")
        hn_loc = persist.tile([P, MT, F], bf16, name="hn_loc")
        hn_scaled = persist.tile([P, MT, FP], bf16, name="hn_scaled")
        xn_loc_b = persist.tile([P, MT, F], bf16, name="xn_loc_b")
        G_b = persist.tile([P, FT, F], bf16, name="G_b")
        GXT = persist.tile([P, MT, F], bf16, name="GXT")
        rowsum_parts = persist.tile([P, MT, NCH], f32, name="rowsum_parts")
        recip_r = persist.tile([P, MT, 1], f32, name="recip_r")
        diag = persist.tile([P, MT, 1], f32, name="diag")
        srow = persist.tile([P, MT, 1], f32, name="srow")
        stdv = persist.tile([P, MT, 1], f32, name="stdv")
        sumsq = persist.tile([P, MT, 1], f32, name="sumsq")
        s_f = persist.tile([P, FT, 1], f32, name="s_f")
        s_b = persist.tile([P, FT, 1], bf16, name="s_b")
        i_cols = persist.tile([P, MT, 4], f32, name="i_cols")
        i_T = persist.tile([P, R], f32r, name="i_T")
        rs_sb = persist.tile([P, MT, FP], f32, name="rs_sb")
        h_agg = persist.tile([P, MT, F], f32, name="h_agg")
        h_aggT = persist.tile([P, FT, R], f32r, name="h_aggT")

        partial_dram = dram.tile([N, FP], f32, name="partial_dram")
        rs_dram = dram.tile([R, FP], f32, name="rs_dram")

        # ============ phase 1: preprocessing + S/E ============
        with tc.tile_pool(name="p1", bufs=1, space="PSUM") as p1, \
             tc.tile_pool(name="sc1", bufs=1) as sc1:

            # bvec = beta @ w_v[:, :F].T into row 3 of wvT3 (rows 0..2 zero),
            # then the w_v tail rows overwrite rows 0..2.
            ps_bv = p1.tile([4, F], f32, name="ps_bv", tag="mm", bufs=4)
            for k in range(FT):
                nc.tensor.matmul(
                    ps_bv[:],
                    bet_pad[:, k],
                    wvT[:, k],
                    start=(k == 0),
                    stop=(k == FT - 1),
                )
            nc.vector.tensor_copy(out=wvT3[:4, :], in_=ps_bv[:])
            nc.sync.dma_start(wvT3[:3, :], wvT_ext[F:])

            # ---- local rows: batched LN/L1 stats ----
            hl_in = sc1.tile([P, MT, F], f32, name="hl_in")
            nc.sync.dma_start(hl_in[:], hloc_v[:])
            xl_in = sc1.tile([P, MT, F], f32, name="xl_in")
            nc.sync.dma_start(xl_in[:], xloc_v[:])
            st6l = small.tile([P, MT, 6], f32, name="st6l", tag="st6b")
            for j in range(MT):
                nc.vector.bn_stats(st6l[:, j], hl_in[:, j])
            mvl = small.tile([P, MT, 2], f32, name="mvl", tag="mvb")
            for j in range(MT):
                nc.vector.bn_aggr(mvl[:, j], st6l[:, j])
            sdl = small.tile([P, MT], f32, name="sdl", tag="sdb")
            nc.scalar.activation(sdl[:], mvl[:, :, 1], AF.Sqrt, bias=eps_ln[:])
            rstdl = small.tile([P, MT], f32, name="rstdl", tag="rstdb")
            nc.vector.reciprocal(rstdl[:], sdl[:])
            nmrl = small.tile([P, MT], f32, name="nmrl", tag="nmrb")
            nc.vector.tensor_tensor(nmrl[:], mvl[:, :, 0], rstdl[:], OP.mult)
            nc.vector.tensor_scalar_mul(nmrl[:], nmrl[:], -1.0)
            l1l = small.tile([P, MT], f32, name="l1l", tag="l1b")
            nc.vector.tensor_reduce(
                l1l[:], xl_in[:], AX.X, OP.add, apply_absolute_value=True
            )
            nc.vector.tensor_scalar_max(l1l[:], l1l[:], L1_EPS)
            rl1l = small.tile([P, MT], f32, name="rl1l", tag="rl1b")
            nc.vector.reciprocal(rl1l[:], l1l[:])
            for mt in range(MT):
                nc.scalar.activation(
                    hn_loc[:, mt], hl_in[:, mt], AF.Identity,
                    bias=nmrl[:, mt : mt + 1], scale=rstdl[:, mt : mt + 1],
                )
                nc.scalar.activation(
                    xn_loc_b[:, mt], xl_in[:, mt], AF.Copy,
                    scale=rl1l[:, mt : mt + 1],
                )
                # diag(a_x)[m] = ||x_m||^2 / l1^2
                sq = small.tile([P, F], f32, name="sq", tag="sq", bufs=2)
                nc.scalar.activation(sq[:], xl_in[:, mt], AF.Square)
                ssq = small.tile([P, 1], f32, name="ssq", tag="ssq")
                nc.vector.tensor_reduce(ssq[:], sq[:], AX.X, OP.add)
                rl1sq = small.tile([P, 1], f32, name="rl1sq", tag="rl1sq")
                nc.scalar.activation(
                    rl1sq[:], rl1l[:, mt : mt + 1], AF.Square
                )
                nc.vector.tensor_tensor(diag[:, mt], ssq[:], rl1sq[:], OP.mult)

            # local transposes (bf16) + kT_loc
            hnT_l = sc1.tile([P, FT, R], bf16, name="hnT_l")
            for ft in range(FT):
                ps_tb = p1.tile([P, 512], bf16, name="ps_tb", tag="tp_b", bufs=2)
                for mt in range(MT):
                    nc.tensor.transpose(
                        ps_tb[:, ts(mt, P)], hn_loc[:, mt, ds(128 * ft, P)], ident_b[:]
                    )
                # gamma/beta are per-partition in this layout
                nc.scalar.activation(
                    hnT_l[:, ft], ps_tb[:], AF.Identity,
                    bias=bet_f[:, ft], scale=gam_f[:, ft],
                )
            for ft in range(FT):
                ps_tb = p1.tile([P, 512], bf16, name="ps_tb2", tag="tp_b", bufs=2)
                for mt in range(MT):
                    nc.tensor.transpose(
                        ps_tb[:, ts(mt, P)], xn_loc_b[:, mt, ds(128 * ft, P)], ident_b[:]
                    )
                nc.vector.tensor_copy(out=xnT_loc[:, ft], in_=ps_tb[:])
            for fo in range(FT):
                ps_k = p1.tile([P, 512], f32, name="ps_k", tag="mm", bufs=4)
                for k in range(FT):
                    nc.tensor.matmul(
                        ps_k[:],
                        wkT[:, k, ds(128 * fo, P)],
                        hnT_l[:, k],
                        start=(k == 0),
                        stop=(k == FT - 1),
                    )
                nc.vector.tensor_copy(out=kT_loc[:, fo], in_=ps_k[:])

            # ---- fused h+x per chunk: LN/L1 -> hnT/xnT -> qT -> S/E -> G ----
            ps_G = p1.tile([P, 2 * F], f32, name="ps_G", tag="gacc", bufs=1)
            for c in range(NCH):
                x_in = stream.tile([P, 4, F], f32, name="x_in", tag="hx_in")
                nc.sync.dma_start(x_in[:], x_v[:, ds(4 * c, 4)])
                l1b = small.tile([P, 4], f32, name="l1x", tag="l1b")
                nc.vector.tensor_reduce(
                    l1b[:], x_in[:], AX.X, OP.add, apply_absolute_value=True
                )
                nc.vector.tensor_scalar_max(l1b[:], l1b[:], L1_EPS)
                rl1b = small.tile([P, 4], f32, name="rl1x", tag="rl1b")
                nc.vector.reciprocal(rl1b[:], l1b[:])
                for j in range(4):
                    nt = 4 * c + j
                    nc.vector.tensor_scalar_mul(
                        xh_b[:, nt, 0:F], x_in[:, j], rl1b[:, j : j + 1]
                    )
                for ft in range(FT):
                    ps_tb = p1.tile([P, 512], bf16, name="ps_tb4", tag="tp_b", bufs=2)
                    for j in range(4):
                        nt = 4 * c + j
                        nc.tensor.transpose(
                            ps_tb[:, ts(j, P)], xh_b[:, nt, ds(128 * ft, P)], ident_b[:]
                        )
                    nc.vector.tensor_copy(out=xnT[:, ft, ds(512 * c, 512)], in_=ps_tb[:])
                for j in range(4):
                    nt = 4 * c + j
                    for m in range(FT):
                        nc.tensor.matmul(
                            ps_G[:, ts(m, F)],
                            xh_b[:, nt, ds(128 * m, P)],
                            xh_b[:, nt, 0:F],
                            start=(nt == 0),
                            stop=(nt == NT - 1),
                        )
                if c == NCH - 1:
                    for m in range(FT):
                        nc.scalar.activation(G_b[:, m], ps_G[:, ts(m, F)], AF.Copy)

                h_in = stream.tile([P, 4, F], f32, name="h_in", tag="hx_in")
                nc.sync.dma_start(h_in[:], h_v[:, ds(4 * c, 4)])
                st6 = small.tile([P, 4, 6], f32, name="st6h", tag="st6b")
                for j in range(4):
                    nc.vector.bn_stats(st6[:, j], h_in[:, j])
                mvb = small.tile([P, 4, 2], f32, name="mvb", tag="mvb")
                for j in range(4):
                    nc.vector.bn_aggr(mvb[:, j], st6[:, j])
                sdb = small.tile([P, 4], f32, name="sdb", tag="sdb")
                nc.scalar.activation(sdb[:], mvb[:, :, 1], AF.Sqrt, bias=eps_ln[:])
                rstdb = small.tile([P, 4], f32, name="rstdb", tag="rstdb")
                nc.vector.reciprocal(rstdb[:], sdb[:])
                nmrb = small.tile([P, 4], f32, name="nmrb", tag="nmrb")
                nc.vector.tensor_tensor(nmrb[:], mvb[:, :, 0], rstdb[:], OP.mult)
                nc.vector.tensor_scalar_mul(nmrb[:], nmrb[:], -1.0)
                rstm = small.tile([P, 4], f32, name="rstm", tag="rstm")
                nc.vector.tensor_tensor(
                    rstm[:], rstdb[:], M01.to_broadcast((P, 4)), OP.mult
                )
                nmrm = small.tile([P, 4], f32, name="nmrm", tag="nmrm")
                nc.vector.tensor_tensor(
                    nmrm[:], nmrb[:], M01.to_broadcast((P, 4)), OP.mult
                )
                for j in range(4):
                    nt = 4 * c + j
                    nc.scalar.activation(
                        xh_b[:, nt, F : 2 * F], h_in[:, j], AF.Identity,
                        bias=nmrm[:, j : j + 1], scale=rstm[:, j : j + 1],
                    )
                hnT_c = sc1.tile([P, FT, 512], bf16, name="hnT_c", tag="hnT_c", bufs=3)
                for ft in range(FT):
                    ps_tb = p1.tile([P, 512], bf16, name="ps_tb3", tag="tp_b", bufs=2)
                    for j in range(4):
                        nt = 4 * c + j
                        nc.tensor.transpose(
                            ps_tb[:, ts(j, P)],
                            xh_b[:, nt, ds(F + 128 * ft, P)],
                            ident_b[:],
                        )
                    nc.vector.tensor_scalar(
                        hnT_c[:, ft], ps_tb[:],
                        scalar1=gam_eff[:, ft], scalar2=bet_f[:, ft],
                        op0=OP.mult, op1=OP.add,
                    )
                for fo in range(FT):
                    ps_q = p1.tile([P, 512], f32, name="ps_q", tag="mm", bufs=4)
                    for k in range(FT):
                        nc.tensor.matmul(
                            ps_q[:],
                            wqT[:, k, ds(128 * fo, P)],
                            hnT_c[:, k],
                            start=(k == 0),
                            stop=(k == FT - 1),
                        )
                    nc.vector.tensor_copy(out=qT[:, fo, ds(512 * c, 512)], in_=ps_q[:])
                # S rows for this chunk, all four local m-tiles
                for mt in range(MT):
                    ps_s = p1.tile([P, 512], f32, name="ps_s", tag="mm", bufs=4)
                    for k in range(FT):
                        nc.tensor.matmul(
                            ps_s[:],
                            kT_loc[:, k, ds(128 * mt, P)],
                            qT[:, k, ds(512 * c, 512)],
                            start=(k == 0),
                            stop=(k == FT - 1),
                        )
                    nc.scalar.activation(
                        E[:, mt, ds(512 * c, 512)],
                        ps_s[:],
                        AF.Exp,
                        scale=SCALE,
                        accum_out=rowsum_parts[:, mt, c : c + 1],
                    )

            # rowsums -> 1/rowsum -> hn_scaled
            for mt in range(MT):
                rs1 = small.tile([P, 1], f32, name="rs1", tag="rs1")
                nc.vector.tensor_reduce(rs1[:], rowsum_parts[:, mt], AX.X, OP.add)
                nc.vector.reciprocal(recip_r[:, mt], rs1[:])
                sch = small.tile([P, 1], f32, name="sch", tag="sch")
                nc.vector.tensor_tensor(sch[:], recip_r[:, mt], M11, OP.mult)
                nc.scalar.activation(
                    hn_scaled[:, mt, 0:F], hn_loc[:, mt], AF.Copy, scale=sch[:]
                )
                nc.vector.memset(hn_scaled[:, mt, F:FP], 0.0)
                nc.vector.tensor_copy(out=hn_scaled[:, mt, F : F + 1], in_=sch[:])

        # ============ phase 2: a_x row stats + bvec ============
        with tc.tile_pool(name="p2", bufs=1, space="PSUM") as p2, \
             tc.tile_pool(name="sc2", bufs=1) as sc2:
            for ft in range(FT):
                nc.vector.tensor_reduce(s_f[:, ft], xnT[:, ft], AX.X, OP.add)
                nc.vector.tensor_copy(out=s_b[:, ft], in_=s_f[:, ft])
            ps_sr = p2.tile([P, MT], f32, name="ps_sr", tag="sr", bufs=1)
            for mt in range(MT):
                for k in range(FT):
                    nc.tensor.matmul(
                        ps_sr[:, mt : mt + 1],
                        xnT_loc[:, k, ds(128 * mt, P)],
                        s_b[:, k],
                        start=(k == 0),
                        stop=(k == FT - 1),
                    )
            nc.vector.tensor_copy(out=srow[:, :, 0], in_=ps_sr[:])

            GX_sb = sc2.tile([P, FT, R], bf16, name="GX_sb")
            for f1 in range(FT):
                ps_gx = p2.tile([P, 512], f32, name="ps_gx", tag="gx", bufs=2)
                for k in range(FT):
                    nc.tensor.matmul(
                        ps_gx[:],
                        G_b[:, k, ds(128 * f1, P)],
                        xnT_loc[:, k],
                        start=(k == 0),
                        stop=(k == FT - 1),
                    )
                nc.vector.tensor_copy(out=GX_sb[:, f1], in_=ps_gx[:])
            for mt in range(MT):
                ps_gxt = p2.tile([P, F], bf16, name="ps_gxt", tag="gxt", bufs=2)
                for f1 in range(FT):
                    nc.tensor.transpose(
                        ps_gxt[:, ts(f1, P)], GX_sb[:, f1, ds(128 * mt, P)], ident_b[:]
                    )
                nc.vector.tensor_copy(out=GXT[:, mt], in_=ps_gxt[:])
            for mt in range(MT):
                tmp = small.tile([P, F], f32, name="tmp_ss", tag="sq", bufs=2)
                nc.vector.tensor_tensor(tmp[:], xn_loc_b[:, mt], GXT[:, mt], OP.mult)
                nc.vector.tensor_reduce(sumsq[:, mt], tmp[:], AX.X, OP.add)
                t1 = small.tile([P, 1], f32, name="t1", tag="t1")
                nc.scalar.activation(t1[:], srow[:, mt], AF.Square)
                nc.vector.tensor_scalar_mul(t1[:], t1[:], -1.0 / N)
                nc.vector.tensor_tensor(t1[:], sumsq[:, mt], t1[:], OP.add)
                nc.vector.tensor_scalar_mul(t1[:], t1[:], 1.0 / (N - 1))
                nc.scalar.activation(stdv[:, mt], t1[:], AF.Sqrt)

        # ============ phase 3: ET/m10, partial+RS, a_xc, combines ============
        with tc.tile_pool(name="pL", bufs=1, space="PSUM") as pL, \
             tc.tile_pool(name="sc3", bufs=1) as sc3:
            ps_b01 = pL.tile([P, 512], f32, name="ps_b01", tag="b01", bufs=1)
            ps_b23 = pL.tile([P, 512], f32, name="ps_b23", tag="b23", bufs=1)
            bslc = [ps_b01[:, 0:F], ps_b01[:, F:], ps_b23[:, 0:F], ps_b23[:, F:]]

            # ---- partial = E.T @ [hn_scaled | m11/rowsum] -> DRAM ----
            for ic in range(NT):
                ps_p = pL.tile([P, FP], f32, name="ps_p", tag="w", bufs=2)
                for jt in range(MT):
                    nc.tensor.matmul(
                        ps_p[:],
                        E[:, jt, ds(128 * ic, P)],
                        hn_scaled[:, jt],
                        start=(jt == 0),
                        stop=(jt == MT - 1),
                    )
                stg = sc3.tile([P, FP], f32, name="stg", tag="stg", bufs=3)
                nc.vector.tensor_copy(out=stg[:], in_=ps_p[:])
                nc.sync.dma_start(
                    partial_dram.rearrange("(a p) f -> p a f", p=P)[:, ic], stg[:]
                )
            # ---- ET transposes + x_out m10 accumulation ----
            for mt in range(MT):
                for c in range(NCH):
                    ps_et = pL.tile([P, 512], bf16, name="ps_et", tag="w", bufs=2)
                    for j in range(4):
                        nt = 4 * c + j
                        nc.tensor.transpose(
                            ps_et[:, ts(j, P)], E[:, mt, ds(128 * nt, P)], ident_b[:]
                        )
                    ET_c = sc3.tile([P, 4, P], bf16, name="ET_c", tag="ET", bufs=3)
                    nc.vector.tensor_copy(out=ET_c[:], in_=ps_et[:])
                    for j in range(4):
                        nt = 4 * c + j
                        nc.tensor.matmul(
                            bslc[mt],
                            ET_c[:, j],
                            xh_b[:, nt, 0:F],
                            start=(nt == 0),
                            stop=(nt == NT - 1),
                        )

            # ---- a_xc streaming + packed [x00 | SYM] accumulation ----
            ps_xs = [
                pL.tile([P, 512], f32, name=f"ps_xs{mt}", tag=f"xs{mt}", bufs=1)
                for mt in range(MT)
            ]
            for nt in range(NT):
                ps_ax = pL.tile([P, 512], f32, name="ps_ax", tag="w", bufs=2)
                for k in range(FT):
                    nc.tensor.matmul(
                        ps_ax[:],
                        xnT[:, k, ds(128 * nt, P)],
                        xnT_loc[:, k],
                        start=(k == 0),
                        stop=(k == FT - 1),
                    )
                axc = sc3.tile([P, 512], bf16, name="axc", tag="axc", bufs=3)
                nc.scalar.activation(axc[:], ps_ax[:], AF.Copy)
                for mt in range(MT):
                    nc.tensor.matmul(
                        ps_xs[mt],
                        axc[:, ds(128 * mt, P)],
                        xh_b[:, nt],
                        start=(nt == 0),
                        stop=(nt == NT - 1),
                    )

            nc.gpsimd.collective_compute(
                "ReduceScatter",
                OP.add,
                replica_groups=[list(range(NCORES))],
                ins=[partial_dram[:]],
                outs=[rs_dram[:]],
            )
            nc.gpsimd.dma_start(rs_sb[:], rs_dram.rearrange("(o p) f -> p o f", p=P))

            # ---- x_out combine ----
            for mt in range(MT):
                xo = small.tile([P, F], f32, name="xo", tag="xo", bufs=2)
                nc.vector.tensor_scalar_mul(xo[:], ps_xs[mt][:, 0:F], M00)
                scb = small.tile([P, 1], f32, name="scb", tag="scb")
                nc.vector.tensor_tensor(scb[:], recip_r[:, mt], M10, OP.mult)
                tb = small.tile([P, F], f32, name="tb", tag="tb", bufs=2)
                nc.scalar.activation(tb[:], bslc[mt], AF.Copy, scale=scb[:])
                nc.vector.tensor_tensor(xo[:], xo[:], tb[:], OP.add)
                xr = stream.tile([P, F], f32, name="xr", tag="res")
                nc.sync.dma_start(xr[:], xloc_v[:, mt])
                nc.vector.tensor_tensor(xo[:], xo[:], xr[:], OP.add)
                nc.sync.dma_start(xout_v[:, mt], xo[:])

            # ---- h path ----
            # i rows: [diag, srow, std, m01*srow + rs_extra] ; h_agg = SYM + rs
            for mt in range(MT):
                nc.gpsimd.tensor_copy(out=i_cols[:, mt, 0:1], in_=diag[:, mt])
                nc.gpsimd.tensor_copy(out=i_cols[:, mt, 1:2], in_=srow[:, mt])
                nc.gpsimd.tensor_copy(out=i_cols[:, mt, 2:3], in_=stdv[:, mt])
                c4 = small.tile([P, 1], f32, name="c4", tag="c4", bufs=4)
                nc.gpsimd.tensor_tensor(c4[:], srow[:, mt], M01, OP.mult)
                nc.gpsimd.tensor_tensor(c4[:], c4[:], rs_sb[:, mt, F : F + 1], OP.add)
                nc.gpsimd.tensor_copy(out=i_cols[:, mt, 3:4], in_=c4[:])
                sym_sb = small.tile([P, F], f32, name="sym_sb", tag="sym", bufs=4)
                nc.vector.tensor_copy(out=sym_sb[:], in_=ps_xs[mt][:, F:])
                nc.gpsimd.tensor_tensor(
                    h_agg[:, mt], sym_sb[:], rs_sb[:, mt, 0:F], OP.add
                )
            nc.vector.memset(i_T[:].bitcast(f32), 0.0)
            for mt in range(MT):
                ps_i = pL.tile([4, P], f32, name="ps_i", tag="w", bufs=2)
                nc.tensor.transpose(ps_i[:], i_cols[:, mt], ident_f[:])
                nc.vector.tensor_copy(out=i_T[:4, ds(128 * mt, P)], in_=ps_i[:])
            for ft in range(FT):
                ps_hat = pL.tile([P, 512], f32, name="ps_hat", tag="w", bufs=2)
                for mt in range(MT):
                    nc.tensor.transpose(
                        ps_hat[:, ts(mt, P)], h_agg[:, mt, ds(128 * ft, P)], ident_f[:]
                    )
                # gamma is a per-partition column scale here
                nc.scalar.activation(h_aggT[:, ft], ps_hat[:], AF.Copy, scale=gam_f[:, ft])
            for mt in range(MT):
                ps_h = pL.tile([P, F], f32, name="ps_h", tag="w", bufs=2)
                for k in range(FT):
                    nc.tensor.matmul(
                        ps_h[:],
                        h_aggT[:, k, ds(128 * mt, P)],
                        wvT[:, k],
                        start=(k == 0),
                        stop=False,
                    )
                nc.tensor.matmul(
                    ps_h[:],
                    i_T[:, ds(128 * mt, P)],
                    wvT3[:],
                    start=False,
                    stop=True,
                )
                vmin = small.tile([P, F], f32, name="vmin", tag="vmin", bufs=2)
                nc.vector.tensor_scalar_min(vmin[:], ps_h[:], 0.0)
                ev = small.tile([P, F], f32, name="ev", tag="ev", bufs=2)
                nc.scalar.activation(ev[:], vmin[:], AF.Exp)
                vmax = small.tile([P, F], f32, name="vmax", tag="vmax", bufs=2)
                nc.vector.tensor_scalar_max(vmax[:], ps_h[:], 0.0)
                ho = small.tile([P, F], f32, name="ho", tag="ho", bufs=2)
                nc.vector.tensor_tensor(ho[:], ev[:], vmax[:], OP.add)
                nc.vector.tensor_scalar_add(ho[:], ho[:], -1.0)
                hr = stream.tile([P, F], f32, name="hr", tag="res")
                nc.sync.dma_start(hr[:], hloc_v[:, mt])
                nc.vector.tensor_tensor(ho[:], ho[:], hr[:], OP.add)
                nc.sync.dma_start(hout_v[:, mt], ho[:])

    nc.finalize()
    return nc


def _make_in_maps(inputs):
    h = np.ascontiguousarray(inputs["h"], dtype=np.float32)
    x = np.ascontiguousarray(inputs["x"], dtype=np.float32)
    w_kT = np.ascontiguousarray(np.asarray(inputs["w_k"], np.float32).T)
    w_qT = np.ascontiguousarray(np.asarray(inputs["w_q"], np.float32).T)
    w_vT = np.ascontiguousarray(np.asarray(inputs["w_v"], np.float32).T)
    mixing = np.ascontiguousarray(inputs["mixing"], dtype=np.float32)
    gam = np.ascontiguousarray(inputs["ln_gamma"], dtype=np.float32)
    bet = np.ascontiguousarray(inputs["ln_beta"], dtype=np.float32)
    return [
        {
            "h": h,
            "x": x,
            "h_loc": np.ascontiguousarray(h[c * R : (c + 1) * R]),
            "x_loc": np.ascontiguousarray(x[c * R : (c + 1) * R]),
            "w_kT": w_kT,
            "w_qT": w_qT,
            "w_vT": w_vT,
            "mixing": mixing,
            "ln_gamma": gam,
            "ln_beta": bet,
        }
        for c in range(NCORES)
    ]


def kernel(h, x, w_k, w_q, w_v, mixing, ln_gamma, ln_beta):
    from concourse.bass_utils import run_bass_kernel_spmd

    if "nc" not in _CACHE:
        _CACHE["nc"] = _build()
    nc = _CACHE["nc"]

    in_maps = _make_in_maps(
        {
            "h": h,
            "x": x,
            "w_k": w_k,
            "w_q": w_q,
            "w_v": w_v,
            "mixing": mixing,
            "ln_gamma": ln_gamma,
            "ln_beta": ln_beta,
        }
    )
    res = run_bass_kernel_spmd(nc, in_maps, list(range(NCORES))).results
    h_out = np.concatenate([res[c]["h_out"] for c in range(NCORES)], axis=0)
    x_out = np.concatenate([res[c]["x_out"] for c in range(NCORES)], axis=0)
    return (h_out, x_out)

